# revision 5
# baseline (speedup 1.0000x reference)
"""JIIF implicit-upsampling MLP on 8 Trainium2 NeuronCores — v2.

Key changes vs v1 baseline:
  * Layer-0 feat/lr matmuls replaced by a per-pixel precomputed table:
    P[pix] = feat[pix]@W0a - lr[pix]@W0c - (pixcoord*64)@D + b0  (bf16,
    [4097, 1024], row 4096 = b0 for out-of-range points).  Each query point
    gathers its 1024-dim L0 partial directly (~8x pixel reuse makes the
    precompute negligible), so per-shift L0 compute collapses to one
    elementwise add + relu.
  * All gathers use dma_gather(transpose=True) on bf16 tables, which lands
    data channel-major [128ch, pts] — no PE transposes anywhere.
  * The shift-independent part (hr_guide chunk + (coord*64)@D + b0) is
    computed once per 512-point tile (HRPC), channel-major, via 8x2 matmuls.
  * Whole MLP in bf16 (fp32 PSUM accumulation); rel-coord cancellation is
    kept exact by baking the same bf16-rounded D into both the P table and
    the on-device cs@D matmul.
  * hr validity masks are replicated across partitions with a K=1 matmul
    (ones[1,128]^T @ maskrow[1,512]) instead of point-major mask tiles.
"""
import sys

if "/opt/trn_rl_repo" not in sys.path:
    sys.path.insert(0, "/opt/trn_rl_repo")

import numpy as np
import ml_dtypes

import concourse.bass as bass
import concourse.bacc as bacc
import concourse.tile as tile
from concourse import mybir
from concourse.masks import make_identity

F32 = mybir.dt.float32
BF16 = mybir.dt.bfloat16
I16 = mybir.dt.int16
OP = mybir.AluOpType
ACTF = mybir.ActivationFunctionType
AX = mybir.AxisListType

MAGIC = 12582912.0  # 1.5 * 2**23 : forces round-to-nearest-even on f32 add
BF = ml_dtypes.bfloat16

B, NFULL = 4, 65536
H_LR = 64
H_HR = 256
NCORES = 8
NP = (B * NFULL) // NCORES  # 32768 points per core
PIX_FL = H_LR * H_LR        # 4096
PIX_HR = H_HR * H_HR        # 65536
SHIFTS = [(-1.0 / 64, -1.0 / 64), (-1.0 / 64, 1.0 / 64),
          (1.0 / 64, -1.0 / 64), (1.0 / 64, 1.0 / 64)]


def build_program(npoints=NP, reps=1):
    assert npoints % 512 == 0
    NQ = npoints // 128          # free-dim length of point-major tiles
    T = NQ // 4                  # number of 512-point tiles

    nc = bacc.Bacc("TRN2", target_bir_lowering=False, debug=False)

    tbl_p = nc.dram_tensor("tbl_p", [PIX_FL + 1, 1024], BF16, kind="ExternalInput")
    tbl_hr_lo = nc.dram_tensor("tbl_hr_lo", [PIX_HR // 2, 128], BF16,
                               kind="ExternalInput")
    tbl_hr_hi = nc.dram_tensor("tbl_hr_hi", [PIX_HR // 2, 128], BF16,
                               kind="ExternalInput")
    coord = nc.dram_tensor("coord", [npoints, 2], F32, kind="ExternalInput")
    w0b = nc.dram_tensor("w0b", [128, 1024], BF16, kind="ExternalInput")
    dsc = nc.dram_tensor("dsc", [2, 1024], BF16, kind="ExternalInput")
    w1 = nc.dram_tensor("w1", [128, 4096], BF16, kind="ExternalInput")
    w2 = nc.dram_tensor("w2", [128, 1024], BF16, kind="ExternalInput")
    w3 = nc.dram_tensor("w3", [128, 256], BF16, kind="ExternalInput")
    w4 = nc.dram_tensor("w4", [128, 2], BF16, kind="ExternalInput")
    bias1 = nc.dram_tensor("bias1", [128, 4], F32, kind="ExternalInput")
    bias2 = nc.dram_tensor("bias2", [128, 2], F32, kind="ExternalInput")
    bias3 = nc.dram_tensor("bias3", [128, 1], F32, kind="ExternalInput")
    bias4 = nc.dram_tensor("bias4", [128, 1], F32, kind="ExternalInput")
    out = nc.dram_tensor("out", [npoints], F32, kind="ExternalOutput")

    evac_ctr = [0]

    def evac_relu(dst, src, bias_ap):
        if evac_ctr[0] % 2 == 0:
            nc.vector.tensor_scalar(dst, src, bias_ap, 0.0, OP.add, OP.max)
        else:
            nc.scalar.activation(dst, src, ACTF.Relu, bias=bias_ap, scale=1.0)
        evac_ctr[0] += 1

    def evac_copy(dst, src):
        if evac_ctr[0] % 2 == 0:
            nc.vector.tensor_copy(dst, src)
        else:
            nc.scalar.copy(dst, src)
        evac_ctr[0] += 1

    with tile.TileContext(nc) as tc:
        with tc.tile_pool(name="const", bufs=1) as cp, \
             tc.tile_pool(name="prol", bufs=1) as pp, \
             tc.tile_pool(name="gat", bufs=3) as gp, \
             tc.tile_pool(name="rhs", bufs=3) as rp, \
             tc.tile_pool(name="act", bufs=2) as ap, \
             tc.tile_pool(name="sm", bufs=2) as smp, \
             tc.tile_pool(name="ps", bufs=1, space="PSUM") as ps:

            ident = cp.tile([128, 128], F32)
            make_identity(nc, ident[:])
            ones1 = cp.tile([1, 128], BF16)
            nc.vector.memset(ones1[:], 1.0)

            # ---- load weights / biases ----
            w0b_s = cp.tile([128, 1024], BF16)
            dsc_s = cp.tile([2, 1024], BF16)
            w1_s = cp.tile([128, 4096], BF16)
            w2_s = cp.tile([128, 1024], BF16)
            w3_s = cp.tile([128, 256], BF16)
            w4_s = cp.tile([128, 2], BF16)
            b1_s = cp.tile([128, 4], F32)
            b2_s = cp.tile([128, 2], F32)
            b3_s = cp.tile([128, 1], F32)
            b4_s = cp.tile([128, 1], F32)
            for dst, src in [(w0b_s, w0b), (dsc_s, dsc), (w1_s, w1),
                             (w2_s, w2), (w3_s, w3), (w4_s, w4),
                             (b1_s, bias1), (b2_s, bias2), (b3_s, bias3),
                             (b4_s, bias4)]:
                nc.sync.dma_start(dst[:], src[:])

            # ---- load coords: point n -> (partition n%128, free n//128) ----
            C = pp.tile([128, NQ, 2], F32)
            nc.sync.dma_start(C[:], coord[:].rearrange("(q p) t -> p q t", p=128))

            # ---- index math (identical to v1) ----
            def axis_index(c_ap, shift, Hval, tag):
                t0 = pp.tile([128, NQ], F32, tag="ax_t")
                if shift is not None:
                    nc.vector.tensor_scalar(t0[:], c_ap, shift, None, OP.add)
                    src = t0[:]
                else:
                    src = c_ap
                v = pp.tile([128, NQ], F32, tag="ax_v")
                nc.vector.tensor_scalar(v[:], src, 1.0, float(Hval), OP.add, OP.mult)
                nc.vector.tensor_scalar(v[:], v[:], 1.0, 0.5, OP.subtract, OP.mult)
                r = pp.tile([128, NQ], F32, tag="ax_r")
                nc.vector.tensor_scalar(r[:], v[:], MAGIC, MAGIC, OP.add, OP.subtract)
                rc = pp.tile([128, NQ], F32, tag=tag[-1] + "_rc")
                nc.vector.tensor_scalar(rc[:], r[:], 0.0, float(Hval - 1), OP.max, OP.min)
                m = pp.tile([128, NQ], F32, tag=tag[-1] + "_m")
                nc.vector.tensor_tensor(m[:], r[:], rc[:], OP.is_equal)
                return rc, m

            def lin_index(ry, rx, my, mx, Hval, tag, redirect=True):
                m = pp.tile([128, NQ], F32, tag="li_mm")
                nc.vector.tensor_tensor(m[:], my[:], mx[:], OP.mult)
                idx = pp.tile([128, NQ], F32, tag="li_idx")
                nc.vector.scalar_tensor_tensor(idx[:], ry[:], float(Hval), rx[:],
                                               OP.mult, OP.add)
                if redirect:
                    zr = float(Hval * Hval)
                    nc.vector.scalar_tensor_tensor(idx[:], idx[:], -zr, m[:],
                                                   OP.add, OP.mult)
                    nc.vector.tensor_scalar(idx[:], idx[:], zr, None, OP.add)
                return idx, m

            def wrap16(src_i16, tag):
                wr = pp.tile([128, NQ * 8], I16, tag=tag + "_wr")
                for ph in range(8):
                    nc.sync.dma_start(wr[0:16, ph::8],
                                      src_i16[ph * 16:(ph + 1) * 16, :])
                for rep in range(1, 8):
                    nc.sync.dma_start(wr[rep * 16:(rep + 1) * 16, :], wr[0:16, :])
                return wr

            cy, cx = C[:, :, 0], C[:, :, 1]

            ry_h, my_h = axis_index(cy, None, H_HR, "hy")
            rx_h, mx_h = axis_index(cx, None, H_HR, "hx")
            idx_hf, m_hr = lin_index(ry_h, rx_h, my_h, mx_h, H_HR, "h",
                                     redirect=False)
            HALF = float(PIX_HR // 2)
            hi_m = pp.tile([128, NQ], F32)
            nc.vector.tensor_scalar(hi_m[:], idx_hf[:], HALF, None, OP.is_ge)
            one_m_hi = pp.tile([128, NQ], F32)
            nc.vector.tensor_scalar(one_m_hi[:], hi_m[:], -1.0, 1.0, OP.mult, OP.add)
            ilo = pp.tile([128, NQ], F32)
            nc.vector.tensor_tensor(ilo[:], idx_hf[:], one_m_hi[:], OP.mult)
            ihi = pp.tile([128, NQ], F32)
            nc.vector.scalar_tensor_tensor(ihi[:], idx_hf[:], -HALF, hi_m[:],
                                           OP.add, OP.mult)
            mlo_m = pp.tile([128, NQ], F32)
            nc.vector.tensor_tensor(mlo_m[:], one_m_hi[:], m_hr[:], OP.mult)
            mhi_m = pp.tile([128, NQ], F32)
            nc.vector.tensor_tensor(mhi_m[:], hi_m[:], m_hr[:], OP.mult)
            ilo16 = pp.tile([128, NQ], I16)
            nc.vector.tensor_copy(ilo16[:], ilo[:])
            ihi16 = pp.tile([128, NQ], I16)
            nc.vector.tensor_copy(ihi16[:], ihi[:])
            wr_hlo = wrap16(ilo16, "hlo")
            wr_hhi = wrap16(ihi16, "hhi")

            idx_fl = []
            for s, (sy, sx) in enumerate(SHIFTS):
                ry, my = axis_index(cy, sy, H_LR, "fy")
                rx, mx = axis_index(cx, sx, H_LR, "fx")
                fidx, m = lin_index(ry, rx, my, mx, H_LR, f"f{s}")
                f16 = pp.tile([128, NQ], I16, tag="f16")
                nc.vector.tensor_copy(f16[:], fidx[:])
                idx_fl.append(wrap16(f16, f"fw{s}"))

            # point-major (cs0, cs1, mlo, mhi) for the per-tile transpose
            pm4 = pp.tile([128, NQ, 4], F32)
            nc.vector.tensor_scalar(pm4[:, :, 0], cy, 64.0, None, OP.mult)
            nc.vector.tensor_scalar(pm4[:, :, 1], cx, 64.0, None, OP.mult)
            nc.vector.tensor_copy(pm4[:, :, 2], mlo_m[:])
            nc.vector.tensor_copy(pm4[:, :, 3], mhi_m[:])

            out_sb = pp.tile([128, NQ], F32)

            # ---- main loop over 512-point tiles ----
            for t in [tt for _ in range(reps) for tt in range(T)]:
                q4 = slice(t * 4, t * 4 + 4)
                w32 = slice(t * 32, (t + 1) * 32)

                # hr gathers, channel-major [128ch, 512pts]
                ghl = gp.tile([128, 1, 512], BF16, tag="ghl")
                ghi = gp.tile([128, 1, 512], BF16, tag="ghi")
                nc.gpsimd.dma_gather(ghl[:], tbl_hr_lo[:], wr_hlo[:, w32],
                                     num_idxs=512, num_idxs_reg=512,
                                     elem_size=128, transpose=True)
                nc.gpsimd.dma_gather(ghi[:], tbl_hr_hi[:], wr_hhi[:, w32],
                                     num_idxs=512, num_idxs_reg=512,
                                     elem_size=128, transpose=True)

                # csT + mask rows, each transposed to a partition-0-based
                # tile (matmul operands must start at partition 0)
                psCS = ps.tile([2, 512], F32, tag="pmm", bufs=3)
                psML = ps.tile([1, 512], F32, tag="msk", bufs=2)
                psMH = ps.tile([1, 512], F32, tag="msk", bufs=2)
                for q in range(4):
                    qs = slice(q * 128, (q + 1) * 128)
                    nc.tensor.transpose(psCS[:, qs], pm4[:, t * 4 + q, 0:2],
                                        ident[:])
                    nc.tensor.transpose(psML[:, qs], pm4[:, t * 4 + q, 2:3],
                                        ident[:])
                    nc.tensor.transpose(psMH[:, qs], pm4[:, t * 4 + q, 3:4],
                                        ident[:])
                cst = rp.tile([2, 512], BF16, tag="cst")
                evac_copy(cst[:], psCS[:])
                mlr = rp.tile([1, 512], BF16, tag="mlr")
                evac_copy(mlr[:], psML[:])
                mhr = rp.tile([1, 512], BF16, tag="mhr")
                evac_copy(mhr[:], psMH[:])

                # replicate mask rows across 128 partitions via K=1 matmul
                mrl = ps.tile([128, 512], F32, tag="pmm", bufs=3)
                nc.tensor.matmul(mrl[:], ones1[:], mlr[:], start=True, stop=True)
                mrh = ps.tile([128, 512], F32, tag="pmm", bufs=3)
                nc.tensor.matmul(mrh[:], ones1[:], mhr[:], start=True, stop=True)

                hrm1 = rp.tile([128, 512], BF16, tag="hrm1")
                nc.vector.tensor_tensor(hrm1[:], ghl[:, 0, :], mrl[:], OP.mult)
                hrm2 = rp.tile([128, 512], BF16, tag="hrm2")
                nc.vector.tensor_tensor(hrm2[:], ghi[:, 0, :], mrh[:], OP.mult)
                hrcm = rp.tile([128, 512], BF16, tag="hrcm")
                nc.vector.tensor_tensor(hrcm[:], hrm1[:], hrm2[:], OP.add)

                # HRPC: shift-independent L0 partial, channel-major bf16
                hrpc = rp.tile([128, 8, 512], BF16, tag="hrpc")
                for m in range(8):
                    ms = slice(m * 128, (m + 1) * 128)
                    pH = ps.tile([128, 512], F32, tag="pmm", bufs=3)
                    nc.tensor.matmul(pH[:], w0b_s[:, ms], hrcm[:],
                                     start=True, stop=False)
                    nc.tensor.matmul(pH[:], dsc_s[:, ms], cst[:],
                                     start=False, stop=True)
                    evac_copy(hrpc[:, m, :], pH[:])

                p4 = ps.tile([128, 32], F32, tag="p4", bufs=2)

                for s in range(4):
                    pg = gp.tile([128, 8, 512], BF16, tag="pg")
                    nc.gpsimd.dma_gather(pg[:], tbl_p[:], idx_fl[s][:, w32],
                                         num_idxs=512, num_idxs_reg=512,
                                         elem_size=1024, transpose=True)

                    # per-chunk add+relu, DVE/ACT pipelined so L1 matmuls can
                    # start as soon as chunk 0 is ready
                    a0p = ap.tile([128, 8, 512], BF16, tag="a0p")
                    a0 = ap.tile([128, 8, 512], BF16, tag="a0")
                    for k in range(8):
                        nc.vector.tensor_tensor(a0p[:, k, :], pg[:, k, :],
                                                hrpc[:, k, :], OP.add)
                        nc.scalar.activation(a0[:, k, :], a0p[:, k, :],
                                             ACTF.Relu)

                    # L1: 1024 -> 512
                    a1 = ap.tile([128, 4, 512], BF16, tag="a1")
                    for m in range(4):
                        p1 = ps.tile([128, 512], F32, tag="pmm", bufs=3)
                        for k in range(8):
                            nc.tensor.matmul(
                                p1[:],
                                w1_s[:, k * 512 + m * 128: k * 512 + (m + 1) * 128],
                                a0[:, k, :],
                                start=(k == 0), stop=(k == 7))
                        evac_relu(a1[:, m, :], p1[:], b1_s[:, m:m + 1])

                    # L2: 512 -> 256
                    a2 = ap.tile([128, 2, 512], BF16, tag="a2")
                    for m in range(2):
                        p2 = ps.tile([128, 512], F32, tag="pmm", bufs=3)
                        for k in range(4):
                            nc.tensor.matmul(
                                p2[:],
                                w2_s[:, k * 256 + m * 128: k * 256 + (m + 1) * 128],
                                a1[:, k, :],
                                start=(k == 0), stop=(k == 3))
                        evac_relu(a2[:, m, :], p2[:], b2_s[:, m:m + 1])

                    # L3: 256 -> 128
                    a3 = ap.tile([128, 512], BF16, tag="a3")
                    p3 = ps.tile([128, 512], F32, tag="pmm", bufs=3)
                    for k in range(2):
                        nc.tensor.matmul(p3[:],
                                         w3_s[:, k * 128:(k + 1) * 128],
                                         a2[:, k, :],
                                         start=(k == 0), stop=(k == 1))
                    evac_relu(a3[:], p3[:], b3_s[:, 0:1])

                    # L4: 128 -> 2, activations stationary -> [pts, 2] in PSUM
                    for q in range(4):
                        off = (q * 4 + s) * 2
                        nc.tensor.matmul(p4[:, off:off + 2],
                                         a3[:, q * 128:(q + 1) * 128],
                                         w4_s[:],
                                         start=True, stop=True)

                # softmax over shifts + weighted sum (point-major layout)
                p4v = p4[:].rearrange("p (q s c) -> p q s c", q=4, s=4)
                mx = smp.tile([128, 4], F32, tag="mx")
                nc.vector.tensor_reduce(mx[:], p4v[:, :, :, 1], AX.X, OP.max)
                e = smp.tile([128, 4, 4], F32, tag="e")
                mxb = mx[:].unsqueeze(2).to_broadcast([128, 4, 4])
                nc.vector.tensor_tensor(e[:], p4v[:, :, :, 1], mxb, OP.subtract)
                nc.scalar.activation(e[:], e[:], ACTF.Exp)
                ssum = smp.tile([128, 4], F32, tag="ssum")
                nc.vector.tensor_reduce(ssum[:], e[:], AX.X, OP.add)
                nc.vector.tensor_tensor(e[:], e[:], p4v[:, :, :, 0], OP.mult)
                num = smp.tile([128, 4], F32, tag="num")
                nc.vector.tensor_reduce(num[:], e[:], AX.X, OP.add)
                rec = smp.tile([128, 4], F32, tag="rec")
                nc.vector.reciprocal(rec[:], ssum[:])
                nc.vector.tensor_tensor(num[:], num[:], rec[:], OP.mult)
                nc.vector.tensor_scalar(out_sb[:, q4], num[:], b4_s[:, 0:1], None,
                                        OP.add)

            nc.sync.dma_start(out[:].rearrange("(q p) -> p q", p=128), out_sb[:])

    nc.compile()
    return nc


def make_in_maps(feat, coord, hr_guide, lr_guide,
                 W0, b0, W1, b1, W2, b2, W3, b3, W4, b4,
                 npoints=NP, ncores=NCORES):
    """Host-side shard + repack. Returns per-core input dicts."""
    f32 = np.float32
    W0 = np.asarray(W0, f32)
    A = W0[0:128]                      # feat part (baked into P)
    BC = W0[128:256] + W0[256:384]     # hr part (on-device)
    Cm = -W0[256:384]                  # lr part (baked into P, negated)
    D = W0[384:386]                    # rel part
    # bf16-rounded D, used BOTH in the P bake and on-device so the large
    # coord/pixel contributions cancel exactly.
    dsc = D.astype(BF)                 # [2, 1024] bf16
    D_b = dsc.astype(f32)

    w0b = np.ascontiguousarray(BC).astype(BF)
    w1r = np.ascontiguousarray(
        np.asarray(W1, f32).reshape(8, 128, 512).transpose(1, 0, 2)
        .reshape(128, 4096)).astype(BF)
    w2r = np.ascontiguousarray(
        np.asarray(W2, f32).reshape(4, 128, 256).transpose(1, 0, 2)
        .reshape(128, 1024)).astype(BF)
    w3r = np.ascontiguousarray(
        np.asarray(W3, f32).reshape(2, 128, 128).transpose(1, 0, 2)
        .reshape(128, 256)).astype(BF)
    w4r = np.ascontiguousarray(np.asarray(W4, f32)).astype(BF)
    b1r = np.ascontiguousarray(np.asarray(b1, f32).reshape(4, 128).T)
    b2r = np.ascontiguousarray(np.asarray(b2, f32).reshape(2, 128).T)
    b3r = np.ascontiguousarray(np.asarray(b3, f32).reshape(1, 128).T)
    b4r = np.full((128, 1), np.asarray(b4, f32)[0], f32)
    b0 = np.asarray(b0, f32)

    # pixel-center coords * 64, raster order (iy*64 + ix)
    n = H_LR
    cc = (-1.0 + 1.0 / n) + (2.0 / n) * np.arange(n, dtype=f32)
    yy, xx = np.meshgrid(cc * 64.0, cc * 64.0, indexing="ij")
    pixc64 = np.stack([yy.ravel(), xx.ravel()], axis=1)  # [4096, 2]
    pix_part = pixc64 @ D_b                               # [4096, 1024]

    per_batch = {}
    for bi in range(B):
        fl = np.asarray(feat[bi], f32).reshape(128, PIX_FL).T      # [4096,128]
        lr = np.asarray(lr_guide[bi], f32).reshape(128, PIX_FL).T  # [4096,128]
        P = np.empty((PIX_FL + 1, 1024), f32)
        P[:PIX_FL] = fl @ A + lr @ Cm - pix_part + b0
        P[PIX_FL] = b0
        thr = np.asarray(hr_guide[bi], f32).reshape(128, PIX_HR).T.astype(BF)
        per_batch[bi] = (np.ascontiguousarray(P.astype(BF)),
                         np.ascontiguousarray(thr[:PIX_HR // 2]),
                         np.ascontiguousarray(thr[PIX_HR // 2:]))

    halves = NFULL // npoints  # cores per batch
    in_maps = []
    for c in range(ncores):
        bi = c // halves
        h = c % halves
        tp, thr_lo, thr_hi = per_batch[bi]
        cslice = np.ascontiguousarray(
            np.asarray(coord[bi, h * npoints:(h + 1) * npoints], f32))
        in_maps.append({
            "tbl_p": tp, "tbl_hr_lo": thr_lo, "tbl_hr_hi": thr_hi,
            "coord": cslice,
            "w0b": w0b, "dsc": dsc,
            "w1": w1r, "w2": w2r, "w3": w3r, "w4": w4r,
            "bias1": b1r, "bias2": b2r, "bias3": b3r, "bias4": b4r,
        })
    return in_maps


_CACHE = {}


def _get_program(npoints=NP, reps=1):
    key = (npoints, reps)
    if key not in _CACHE:
        _CACHE[key] = build_program(npoints, reps)
    return _CACHE[key]


def run_on_hw(inputs, trace=False):
    from concourse.bass_utils import run_bass_kernel_spmd
    nc = _get_program(NP)
    in_maps = make_in_maps(**inputs)
    res = run_bass_kernel_spmd(nc, in_maps, list(range(NCORES)), trace=trace)
    out = np.empty((B, NFULL, 1), np.float32)
    halves = NFULL // NP
    for c in range(NCORES):
        bi, h = c // halves, c % halves
        out[bi, h * NP:(h + 1) * NP, 0] = res.results[c]["out"]
    return out, res


def kernel(**inputs):
    out, _ = run_on_hw(inputs, trace=False)
    return out
